# revision 46
# baseline (speedup 1.0000x reference)
"""GCN encoder (2-layer GCNConv, PyG-style symmetric norm w/ self loops) on
8 Trainium2 NeuronCores.

Strategy (node-sharded by destination, edge-cut partitioning):
  out = M @ relu(M @ x @ W1 + b1) @ W2 + b2   with  M = D^-1/2 (A+I) D^-1/2

Key algebra: M (row mixing) commutes with W (column mixing), so layer 1 is
computed aggregation-FIRST on the host-prescaled table x' = dis*x which every
core holds in full -> layer 1 needs ZERO inter-core communication.  Layer 2
transforms the layer-1 output locally (h2' = (dis*out1) @ W2) and AllGathers
the small [N,128] bf16 table, then aggregates.

Aggregation on-chip: dma_gather of source rows (edges sorted by (dest-tile,
source-block)), then a one-hot matmul segment-sum: for each 128-edge column
block build S[e,d] = (dest_local[e]==d) with a DVE is_equal against an iota
tile, and accumulate S.T @ msgs into a PSUM tile of 128 destinations.

All per-core index/dest tables are built on the host (numpy) and passed as
extra ExternalInputs; the compiled program is identical across cores (SPMD).
"""

import math
import os

import numpy as np
import ml_dtypes

BF16 = ml_dtypes.bfloat16

# ---------------------------------------------------------------- problem cfg
N = 100000
E_EDGES = 3200000
IN_C = 256
HID = 256
OUT_C = 128
NCORES = 8


def make_cfg(n_nodes, in_c, hid, out_c, tsup=4, msg_bufs=6, chunk_tiles=8, nb=4):
    msg_bufs = int(os.environ.get("GCN_MSGBUFS", msg_bufs))
    nb = int(os.environ.get("GCN_NB", nb))
    tsup = int(os.environ.get("GCN_TSUP", tsup))
    pn = n_nodes // NCORES
    tpc = (pn + 127) // 128
    ppad = tpc * 128
    bs1 = n_nodes // nb
    # layer-2 window: whole shards (bs1 >= pn) or a sub-shard slice
    if bs1 >= pn:
        assert bs1 % pn == 0
        bs2 = (bs1 // pn) * ppad
    else:
        assert pn % bs1 == 0
        bs2 = bs1
    cfg = dict(
        N=n_nodes,
        IN=in_c,
        HID=hid,
        OUT=out_c,
        PN=pn,
        TPC=tpc,
        PPAD=ppad,
        NB=nb,
        BS1=bs1,
        BS2=bs2,
        TSUP=tsup,
        MSG_BUFS=msg_bufs,
        CHUNK_TILES=chunk_tiles,
    )
    assert n_nodes % 8 == 0
    assert cfg["BS1"] < 32768 and cfg["BS2"] < 32768, "int16 gather index limit"
    return cfg


CFG = make_cfg(N, IN_C, HID, OUT_C)


# ---------------------------------------------------------------- host prep
def prep_inputs(cfg, x, edge_index, W1, b1, W2, b2):
    """Shard/encode everything on the host. Returns (in_maps, meta)."""
    n, pn, tpc, ppad, nb = cfg["N"], cfg["PN"], cfg["TPC"], cfg["PPAD"], cfg["NB"]
    bs1 = cfg["BS1"]
    in_c, hid, out_c = cfg["IN"], cfg["HID"], cfg["OUT"]

    x = np.asarray(x, np.float32)
    edge_index = np.asarray(edge_index)
    row = edge_index[0].astype(np.int64)
    col = edge_index[1].astype(np.int64)

    deg = np.bincount(col, minlength=n).astype(np.float32) + 1.0
    dis = 1.0 / np.sqrt(deg)  # [n]

    xp = (x * dis[:, None]).astype(BF16)  # gather table for layer 1

    clamp = int(os.environ.get("GCN_CLAMP", "0"))
    if clamp:
        # timing-only probe: shrink the random-access window of the L1 gather
        row = row % (n // 4 // clamp)

    core = col // pn
    t_of = (col - core * pn) // 128
    dl = (col - core * pn) % 128  # dest local within tile
    b_of = row // bs1
    l1i = (row - b_of * bs1).astype(np.int16)
    shard = row // pn
    if bs1 >= pn:
        spb = bs1 // pn
        l2i = ((shard - spb * b_of) * ppad + (row - shard * pn)).astype(np.int16)
    else:
        bps = pn // bs1
        l2i = (row - shard * pn - (b_of % bps) * bs1).astype(np.int16)

    # counts per (core, tile, block)
    key = (core * tpc + t_of) * nb + b_of
    counts = np.bincount(key, minlength=NCORES * tpc * nb).reshape(NCORES, tpc, nb)
    seg_len = ((counts.max(axis=0) + 127) // 128) * 128  # [tpc, nb]

    # global layout order: (super, block, tile-within-super)
    tsup = cfg["TSUP"]
    supers = [list(range(i, min(i + tsup, tpc))) for i in range(0, tpc, tsup)]
    off = np.zeros((tpc, nb), np.int64)
    nidx = np.zeros((len(supers), nb), np.int64)
    co = np.zeros((len(supers), nb), np.int64)
    pos = 0
    for si, s in enumerate(supers):
        for b in range(nb):
            co[si][b] = pos
            for t in s:
                off[t, b] = pos
                pos += seg_len[t, b]
            nidx[si][b] = pos - co[si][b]
    tot = pos
    assert tot % 128 == 0
    # per-call gather size quantized to 256 so only a handful of distinct
    # num_idxs_reg values exist (each distinct value costs a Pool register)
    nidx_call = ((nidx + 255) // 256) * 256
    tot_io = tot + 256  # idx arrays get a zero tail so quantized reads stay in bounds

    # per-edge positions: sort by (core, tile, block), place at off + rank
    sort_key = key
    order = np.argsort(sort_key, kind="stable")
    ranks = np.empty(len(order), np.int64)
    # rank within group = position - start of group
    group_start = np.zeros(NCORES * tpc * nb + 1, np.int64)
    np.cumsum(np.bincount(sort_key, minlength=NCORES * tpc * nb), out=group_start[1:])
    ranks[order] = np.arange(len(order)) - group_start[sort_key[order]]
    pos_of_edge = off[t_of, b_of] + ranks

    in_maps = []
    xp128 = np.ascontiguousarray(xp[:, :128]) if os.environ.get("GCN_TAB128") else None
    w1c = np.ascontiguousarray(W1.astype(BF16).reshape(2, 128, hid))
    w2c = np.ascontiguousarray(W2.astype(BF16).reshape(2, 128, out_c))
    b1r = np.ascontiguousarray(np.tile(np.asarray(b1, np.float32)[None, :], (128, 1)))
    b2r = np.ascontiguousarray(np.tile(np.asarray(b2, np.float32)[None, :], (128, 1)))
    iota = np.ascontiguousarray(
        np.tile(np.arange(128, dtype=np.float32)[None, :], (128, 1)).astype(BF16)
    )

    for c in range(NCORES):
        sel = core == c
        p = pos_of_edge[sel]
        idx1 = np.zeros(tot, np.int16)
        idx2 = np.zeros(tot, np.int16)
        dst = np.full(tot, -1.0, np.float32)
        idx1[p] = l1i[sel]
        idx2[p] = l2i[sel]
        dst[p] = dl[sel].astype(np.float32)

        idx1 = np.concatenate([idx1, np.zeros(tot_io - tot, np.int16)])
        idx2 = np.concatenate([idx2, np.zeros(tot_io - tot, np.int16)])
        idx1_w = np.tile(np.ascontiguousarray(idx1.reshape(-1, 16).T), (8, 1))
        idx2_w = np.tile(np.ascontiguousarray(idx2.reshape(-1, 16).T), (8, 1))
        dst_w = np.ascontiguousarray(dst.reshape(-1, 128).T)

        dis_own = np.ones(ppad, np.float32)
        dis_own[:pn] = dis[c * pn : (c + 1) * pn]
        dis_own_w = np.ascontiguousarray(dis_own.reshape(tpc, 128).T)

        xself = np.zeros((ppad, in_c), BF16)
        xself[:pn] = (
            x[c * pn : (c + 1) * pn] * (dis[c * pn : (c + 1) * pn] ** 2)[:, None]
        ).astype(BF16)

        extra = {"xp128": xp128} if xp128 is not None else {}
        if os.environ.get("GCN_XPAG"):
            extra["xpsh"] = np.ascontiguousarray(xp[c * pn : (c + 1) * pn])
        in_maps.append(
            dict(
                xp=xp,
                xself=xself,
                **extra,
                idx1=idx1_w,
                idx2=idx2_w,
                dst=dst_w,
                diso=dis_own_w,
                w1=w1c,
                w2=w2c,
                b1=b1r,
                b2=b2r,
                iota=iota,
            )
        )

    meta = dict(seg_len=seg_len, off=off, nidx=nidx, co=co, tot=tot, supers=supers,
                nidx_call=nidx_call, tot_io=tot_io)
    return in_maps, meta


# ---------------------------------------------------------------- bass build
def build_program(cfg, meta):
    import concourse.bass as bass
    import concourse.mybir as mybir
    import concourse.tile as tile
    from contextlib import ExitStack

    f32 = mybir.dt.float32
    bf16 = mybir.dt.bfloat16
    i16 = mybir.dt.int16
    Alu = mybir.AluOpType
    Act = mybir.ActivationFunctionType

    n, pn, tpc, ppad, nb = cfg["N"], cfg["PN"], cfg["TPC"], cfg["PPAD"], cfg["NB"]
    bs1, bs2 = cfg["BS1"], cfg["BS2"]
    in_c, hid, out_c = cfg["IN"], cfg["HID"], cfg["OUT"]
    seg_len, off = meta["seg_len"], meta["off"]
    nidx, co, tot, supers = meta["nidx"], meta["co"], meta["tot"], meta["supers"]
    nidx_call, tot_io = meta["nidx_call"], meta["tot_io"]

    import concourse.bacc as bacc

    nq = int(os.environ.get("GCN_NQ", "1"))
    nc = bacc.Bacc(None, num_devices=NCORES, num_swdge_queues=nq)

    xp_d = nc.dram_tensor("xp", [n, in_c], bf16, kind="ExternalInput")
    xp128_d = (
        nc.dram_tensor("xp128", [n, 128], bf16, kind="ExternalInput")
        if os.environ.get("GCN_TAB128")
        else None
    )
    xself_d = nc.dram_tensor("xself", [ppad, in_c], bf16, kind="ExternalInput")
    idx1_d = nc.dram_tensor("idx1", [128, tot_io // 16], i16, kind="ExternalInput")
    idx2_d = nc.dram_tensor("idx2", [128, tot_io // 16], i16, kind="ExternalInput")
    dst_d = nc.dram_tensor("dst", [128, tot // 128], f32, kind="ExternalInput")
    diso_d = nc.dram_tensor("diso", [128, tpc], f32, kind="ExternalInput")
    w1_d = nc.dram_tensor("w1", [2, 128, hid], bf16, kind="ExternalInput")
    w2_d = nc.dram_tensor("w2", [2, 128, out_c], bf16, kind="ExternalInput")
    b1_d = nc.dram_tensor("b1", [128, hid], f32, kind="ExternalInput")
    b2_d = nc.dram_tensor("b2", [128, out_c], f32, kind="ExternalInput")
    iota_d = nc.dram_tensor("iota", [128, 128], bf16, kind="ExternalInput")
    out_d = nc.dram_tensor("out", [ppad, out_c], f32, kind="ExternalOutput")

    # AllGather table (pair-shared HBM output)
    tab_d = nc.dram_tensor("tab", [NCORES * ppad, out_c], bf16, addr_space="Shared")

    chunk_tiles = cfg["CHUNK_TILES"]
    chunks = [
        list(range(i, min(i + chunk_tiles, tpc))) for i in range(0, tpc, chunk_tiles)
    ]

    nreg_cache = {}
    stages = os.environ.get("GCN_STAGES", "all")

    with tile.TileContext(nc) as tc, ExitStack() as ctx:
        def nreg(v):
            if v not in nreg_cache:
                nreg_cache[v] = nc.gpsimd.to_reg(v)
            return nreg_cache[v]
        cpool = ctx.enter_context(tc.tile_pool(name="const", bufs=1))
        iota_t = cpool.tile([128, 128], bf16)
        nc.sync.dma_start(iota_t[:], iota_d[:])
        w1_t = cpool.tile([128, 2, hid], bf16)
        w2_t = cpool.tile([128, 2, out_c], bf16)
        for k in range(2):
            nc.sync.dma_start(w1_t[:, k, :], w1_d[k])
            nc.sync.dma_start(w2_t[:, k, :], w2_d[k])
        b1_t = cpool.tile([128, hid], f32)
        nc.sync.dma_start(b1_t[:], b1_d[:])
        b2_t = cpool.tile([128, out_c], f32)
        nc.sync.dma_start(b2_t[:], b2_d[:])
        diso_t = cpool.tile([128, tpc], f32)
        nc.sync.dma_start(diso_t[:], diso_d[:])

        dram = ctx.enter_context(tc.tile_pool(name="dram", bufs=1, space="DRAM"))
        mx_dm = dram.tile([ppad, hid], bf16)
        o1s_dm = dram.tile([ppad, hid], bf16)
        agin_dm = dram.tile([ppad, out_c], bf16)

        xp_src = xp_d
        xp_copy_deps = None
        if os.environ.get("GCN_XPAG"):
            # table produced by an AllGather collective (mirrors the L2 tab
            # path exactly): each core contributes its own 12500-row shard.
            import bass_rust as _brx

            xpsh_d = nc.dram_tensor("xpsh", [n // NCORES, in_c], bf16,
                                    kind="ExternalInput")
            xp_ag = nc.dram_tensor("xp_ag", [NCORES * (n // NCORES), in_c], bf16,
                                   addr_space="Shared")
            xpsh_i = dram.tile([n // NCORES, in_c], bf16)
            cpx = nc.sync.dma_start(xpsh_i[:], xpsh_d[:])
            ccx = nc.gpsimd.collective_compute(
                "AllGather",
                mybir.AluOpType.bypass,
                replica_groups=[list(range(NCORES))],
                ins=[xpsh_i[:, :].opt()],
                outs=[xp_ag.ap().opt()],
            )
            _brx.add_dep_helper(ccx.ins, cpx.ins, sync=True,
                                reason="shard copy before AG")
            xp_copy_deps = [ccx]
            xp_src = xp_ag
        elif os.environ.get("GCN_COPYXP_SH"):
            xp_l = nc.dram_tensor("xp_sh", [n, in_c], bf16, addr_space="Shared")
            xp_copy_deps = []
            for ci in range(8):
                r0 = ci * (n // 8)
                xp_copy_deps.append(
                    nc.sync.dma_start(
                        xp_l[r0 : r0 + n // 8, :], xp_d[r0 : r0 + n // 8, :]
                    )
                )
            xp_src = xp_l
        elif os.environ.get("GCN_COPYXP"):
            xp_l = dram.tile([n, in_c], bf16)
            xp_copy_deps = []
            for ci in range(8):
                r0 = ci * (n // 8)
                xp_copy_deps.append(
                    nc.sync.dma_start(
                        xp_l[r0 : r0 + n // 8, :], xp_d[r0 : r0 + n // 8, :]
                    )
                )
            xp_src = xp_l

        # ---------------- layer-1 aggregation: Mx = dis*(A @ x') + dis^2*x
        qcounter = [0]

        def aggregate(idx_d, table_d, row0fn, win, feat, consume_tile,
                      extra_dep=None, estep=None):
            """Emit gather + one-hot matmul aggregation; call consume_tile(t,
            psum_ap) for each dest tile's accumulated [128, feat] PSUM."""
            import bass_rust as _br
            with (
                tc.tile_pool(name="idx", bufs=3) as ipool,
                tc.tile_pool(name="msgs", bufs=cfg["MSG_BUFS"]) as mpool,
                tc.tile_pool(name="dstp", bufs=3) as dpool,
                tc.tile_pool(name="oneh", bufs=8) as spool,
                tc.tile_pool(name="psagg", bufs=3, space="PSUM") as pa,
            ):
                maxcols = max(1, int(nidx_call.max()) // 128)
                gathers = []
                for si, s in enumerate(supers):
                    msgs = {}
                    for b in range(nb):
                        ni = int(nidx_call[si][b])
                        if ni == 0:
                            continue
                        it = ipool.tile([128, max(int(nidx_call.max()) // 16, 8)], i16, tag="idx")
                        c0 = int(co[si][b])
                        nc.sync.dma_start(
                            it[:, : ni // 16], idx_d[:, c0 // 16 : (c0 + ni) // 16]
                        )
                        mt = mpool.tile([128, maxcols, feat], bf16, tag="msgs")
                        g = nc.gpsimd.dma_gather(
                            mt[:, : ni // 128, :],
                            table_d[row0fn(b) : row0fn(b) + win, :feat],
                            it[:, : ni // 16],
                            ni,
                            nreg(ni),
                            feat,
                            elem_step=estep or feat,
                            single_packet=False,
                            queue_num=qcounter[0] % nq,
                        )
                        qcounter[0] += 1
                        if extra_dep is not None:
                            for dep in extra_dep:
                                _br.add_dep_helper(
                                    g.ins, dep.ins, sync=True, reason="xp copy"
                                )
                        gathers.append(g)
                        msgs[b] = mt
                    scol0 = int(co[si][0]) // 128
                    scols = int(nidx[si].sum()) // 128
                    if scols == 0:
                        continue
                    dt_ = dpool.tile([128, max(1, int(nidx.sum(1).max()) // 128)],
                                     f32, tag="dst")
                    nc.sync.dma_start(
                        dt_[:, :scols], dst_d[:, scol0 : scol0 + scols]
                    )
                    for t in s:
                        nblk = int(seg_len[t].sum()) // 128
                        if nblk == 0:
                            continue
                        ps = pa.tile([128, feat], f32, tag="psagg")
                        k = 0
                        for b in range(nb):
                            nbt = int(seg_len[t][b]) // 128
                            mcol0 = (int(off[t][b]) - int(co[si][b])) // 128
                            for j in range(nbt):
                                S = spool.tile([128, 128], bf16, tag="oneh")
                                dcol = int(off[t][b]) // 128 + j - scol0
                                if os.environ.get("GCN_SKIP_MM"):
                                    continue
                                nc.vector.tensor_scalar(
                                    S[:],
                                    iota_t[:],
                                    dt_[:, dcol : dcol + 1],
                                    None,
                                    Alu.is_equal,
                                )
                                nc.tensor.matmul(
                                    ps[:],
                                    S[:],
                                    msgs[b][:, mcol0 + j, :],
                                    start=(k == 0),
                                    stop=(k == nblk - 1),
                                )
                                k += 1
                        if not os.environ.get("GCN_SKIP_EPI"):
                            consume_tile(t, ps)
            return gathers

        with (
            tc.tile_pool(name="xself", bufs=3) as xsp,
            tc.tile_pool(name="mxout", bufs=3) as mxp,
        ):
            def consume_l1(t, ps):
                xst = xsp.tile([128, in_c], bf16, tag="xself")
                nc.sync.dma_start(xst[:], xself_d[t * 128 : (t + 1) * 128, :])
                mxt = mxp.tile([128, hid], bf16, tag="mx")
                nc.vector.scalar_tensor_tensor(
                    mxt[:], ps[:], diso_t[:, t : t + 1], xst[:],
                    mybir.AluOpType.mult, mybir.AluOpType.add,
                )
                nc.sync.dma_start(mx_dm[t * 128 : (t + 1) * 128, :], mxt[:])

            if xp128_d is not None:
                aggregate(idx1_d, xp128_d, lambda b: b * bs1, bs1, 128,
                          consume_l1)
            elif os.environ.get("GCN_HALF_FEAT"):
                aggregate(idx1_d, xp_src, lambda b: b * bs1, bs1, in_c // 2,
                          consume_l1, extra_dep=xp_copy_deps, estep=in_c)
            else:
                aggregate(idx1_d, xp_src, lambda b: b * bs1, bs1, in_c,
                          consume_l1, extra_dep=xp_copy_deps)

        # ---------------- W1 matmul + relu + dis scale: out1s = dis*relu(Mx@W1+b1)
        if stages != "l1":
          with (
             tc.tile_pool(name="mxT", bufs=4) as tp,
             tc.tile_pool(name="tadd", bufs=3) as tap,
             tc.tile_pool(name="o1", bufs=3) as o1p,
             tc.tile_pool(name="psB", bufs=2, space="PSUM") as pb,
         ):
             for ch in chunks:
                 r0, rc = ch[0] * 128, len(ch) * 128
                 mxT = []
                 for k in range(2):
                     mt = tp.tile([128, chunk_tiles * 128], bf16, tag="mxT")
                     nc.sync.dma_start_transpose(
                         mt[:, :rc], mx_dm[r0 : r0 + rc, k * 128 : (k + 1) * 128]
                     )
                     mxT.append(mt)
                 for ri, t in enumerate(ch):
                     ps = pb.tile([128, hid], f32, tag="psB")
                     for k in range(2):
                         nc.tensor.matmul(
                             ps[:],
                             mxT[k][:, ri * 128 : (ri + 1) * 128],
                             w1_t[:, k, :],
                             start=(k == 0),
                             stop=(k == 1),
                         )
                     tt = tap.tile([128, hid], f32, tag="tadd")
                     nc.vector.tensor_add(tt[:], ps[:], b1_t[:])
                     o1 = o1p.tile([128, hid], bf16, tag="o1")
                     nc.scalar.activation(
                         o1[:], tt[:], Act.Relu, scale=diso_t[:, t : t + 1]
                     )
                     nc.sync.dma_start(o1s_dm[t * 128 : (t + 1) * 128, :], o1[:])

        # ---------------- W2 matmul: h2' = out1s @ W2  -> AllGather input
        if stages not in ("l1", "l1b"):
          with (
             tc.tile_pool(name="o1T", bufs=4) as tp2,
             tc.tile_pool(name="h2", bufs=3) as h2p,
             tc.tile_pool(name="psC", bufs=2, space="PSUM") as pc,
         ):
             for ch in chunks:
                 r0, rc = ch[0] * 128, len(ch) * 128
                 o1T = []
                 for k in range(2):
                     ot = tp2.tile([128, chunk_tiles * 128], bf16, tag="o1T")
                     nc.sync.dma_start_transpose(
                         ot[:, :rc], o1s_dm[r0 : r0 + rc, k * 128 : (k + 1) * 128]
                     )
                     o1T.append(ot)
                 for ri, t in enumerate(ch):
                     ps = pc.tile([128, out_c], f32, tag="psC")
                     for k in range(2):
                         nc.tensor.matmul(
                             ps[:],
                             o1T[k][:, ri * 128 : (ri + 1) * 128],
                             w2_t[:, k, :],
                             start=(k == 0),
                             stop=(k == 1),
                         )
                     h2 = h2p.tile([128, out_c], bf16, tag="h2")
                     nc.vector.tensor_copy(h2[:], ps[:])
                     nc.sync.dma_start(agin_dm[t * 128 : (t + 1) * 128, :], h2[:])

        # ---------------- AllGather h2' table
        if stages not in ("l1", "l1b", "l1bc"):
          cc = nc.gpsimd.collective_compute(
             "AllGather",
             mybir.AluOpType.bypass,
             replica_groups=[list(range(NCORES))],
             ins=[agin_dm[:, :].opt()],
             outs=[tab_d.ap().opt()],
         )

        # ---------------- layer-2 aggregation + epilogue
        import bass_rust as _br

        if stages not in ("l1", "l1b", "l1bc", "ag"):
          with (
             tc.tile_pool(name="h2own", bufs=3) as hop,
             tc.tile_pool(name="wtmp", bufs=3) as wtp,
             tc.tile_pool(name="o2", bufs=3) as o2p,
         ):
             def consume_l2(t, ps):
                 h2o = hop.tile([128, out_c], bf16, tag="h2own")
                 nc.sync.dma_start(h2o[:], agin_dm[t * 128 : (t + 1) * 128, :])
                 wt = wtp.tile([128, out_c], f32, tag="wtmp")
                 nc.vector.scalar_tensor_tensor(
                     wt[:], h2o[:], diso_t[:, t : t + 1], b2_t[:],
                     mybir.AluOpType.mult, mybir.AluOpType.add,
                 )
                 o2 = o2p.tile([128, out_c], f32, tag="o2")
                 nc.vector.scalar_tensor_tensor(
                     o2[:], ps[:], diso_t[:, t : t + 1], wt[:],
                     mybir.AluOpType.mult, mybir.AluOpType.add,
                 )
                 nc.sync.dma_start(out_d[t * 128 : (t + 1) * 128, :], o2[:])

             pn_, ppad_ = pn, ppad
             if bs1 >= pn_:
                 l2row0 = lambda b: b * bs2
             else:
                 bps = pn_ // bs1
                 l2row0 = lambda b: (b // bps) * ppad_ + (b % bps) * bs1
             gathers2 = aggregate(idx2_d, tab_d, l2row0, bs2, out_c, consume_l2)
             # gathers from tab_d (plain Shared DRAM tensor) must wait for the
             # collective -- DRAM tensor deps aren't tracked by Tile.
             for g in gathers2:
                 _br.add_dep_helper(g.ins, cc.ins, sync=True, reason="tab after AG")

        if stages != "all":
            nc.sync.dma_start(out_d[0:128, :], b2_t[:])

    nc.compile()
    return nc


# ================================================================ V2 path
# HBM dma_gather costs ~31ns/descriptor (HBM round-trip latency per random
# row, no pipelining inside an SDMA engine).  V2 streams each 12500-row
# source block into SBUF sequentially (row-wrapped: row r -> partition r%128,
# rank r//128) and gathers per-edge messages with SBUF-source
# dma_gather(transpose=True), which avoids the HBM latency entirely.  The
# feature-major gather output is transposed back per 128-edge group on the
# TensorEngine (identity matmul), then the existing one-hot segment-sum runs
# unchanged.  Eight passes (one per source block) accumulate partial segment
# sums into DRAM f32 via CCE accumulate-DMA.


def prep_inputs_v2(cfg, x, edge_index, W1, b1, W2, b2):
    n, pn, tpc, ppad, nb = cfg["N"], cfg["PN"], cfg["TPC"], cfg["PPAD"], cfg["NB"]
    bs1 = cfg["BS1"]
    in_c, hid, out_c = cfg["IN"], cfg["HID"], cfg["OUT"]
    assert bs1 == pn, "V2 requires nb=8 (source block == shard)"

    x = np.asarray(x, np.float32)
    edge_index = np.asarray(edge_index)
    row = edge_index[0].astype(np.int64)
    col = edge_index[1].astype(np.int64)

    deg = np.bincount(col, minlength=n).astype(np.float32) + 1.0
    dis = 1.0 / np.sqrt(deg)
    xp = (x * dis[:, None]).astype(BF16)

    core = col // pn
    t_of = (col - core * pn) // 128
    dl = (col - core * pn) % 128
    b_of = row // bs1
    li = (row - b_of * bs1).astype(np.int16)

    key = (core * tpc + t_of) * nb + b_of
    counts = np.bincount(key, minlength=NCORES * tpc * nb).reshape(NCORES, tpc, nb)
    seg_len = ((counts.max(axis=0) + 127) // 128) * 128  # [tpc, nb]

    tsup = cfg["TSUP"]
    supers = [list(range(i, min(i + tsup, tpc))) for i in range(0, tpc, tsup)]
    nsup = len(supers)
    off = np.zeros((tpc, nb), np.int64)
    nidx = np.zeros((nb, nsup), np.int64)
    co = np.zeros((nb, nsup), np.int64)
    pos = 0
    for b in range(nb):
        for si, s in enumerate(supers):
            co[b][si] = pos
            for t in s:
                off[t, b] = pos
                pos += seg_len[t, b]
            nidx[b][si] = pos - co[b][si]
    tot = pos
    assert tot % 128 == 0
    ni_u = int(((nidx.max() + 255) // 256) * 256)  # uniform per-call gather size
    tot_io = tot + ni_u + 256

    order = np.argsort(key, kind="stable")
    ranks = np.empty(len(order), np.int64)
    group_start = np.zeros(NCORES * tpc * nb + 1, np.int64)
    np.cumsum(np.bincount(key, minlength=NCORES * tpc * nb), out=group_start[1:])
    ranks[order] = np.arange(len(order)) - group_start[key[order]]
    pos_of_edge = off[t_of, b_of] + ranks

    if os.environ.get("GCN_APG"):
        # feature-major tables for ap_gather: xpw[b, f, i, j] = xp[b*bs1+i, f+128j]
        xpw = np.ascontiguousarray(
            xp.reshape(nb, bs1, 2, 128).transpose(0, 3, 1, 2)
        )
    else:
        # row-wrapped gather tables: xpw[b, p, s, :] = xp[b*bs1 + s*128 + p]
        xpad = np.zeros((nb, tpc * 128, in_c), BF16)
        for b in range(nb):
            xpad[b, :bs1] = xp[b * bs1 : (b + 1) * bs1]
        xpw = np.ascontiguousarray(
            xpad.reshape(nb, tpc, 128, in_c).transpose(0, 2, 1, 3)
        )

    w1c = np.ascontiguousarray(W1.astype(BF16).reshape(2, 128, hid))
    w2c = np.ascontiguousarray(W2.astype(BF16).reshape(2, 128, out_c))
    b1r = np.ascontiguousarray(np.tile(np.asarray(b1, np.float32)[None, :], (128, 1)))
    b2r = np.ascontiguousarray(np.tile(np.asarray(b2, np.float32)[None, :], (128, 1)))
    iota = np.ascontiguousarray(
        np.tile(np.arange(128, dtype=np.float32)[None, :], (128, 1)).astype(BF16)
    )
    ident = np.ascontiguousarray(np.eye(128, dtype=np.float32).astype(BF16))

    in_maps = []
    for c in range(NCORES):
        sel = core == c
        p = pos_of_edge[sel]
        idx = np.zeros(tot_io, np.int16)
        dst = np.full(tot, -1.0, np.float32)
        idx[p] = li[sel]
        dst[p] = dl[sel].astype(np.float32)
        idx_w = np.tile(np.ascontiguousarray(idx.reshape(-1, 16).T), (8, 1))
        dst_w = np.ascontiguousarray(dst.reshape(-1, 128).T)

        dis_own = np.ones(ppad, np.float32)
        dis_own[:pn] = dis[c * pn : (c + 1) * pn]
        dis_own_w = np.ascontiguousarray(dis_own.reshape(tpc, 128).T)

        xself = np.zeros((ppad, in_c), BF16)
        xself[:pn] = (
            x[c * pn : (c + 1) * pn] * (dis[c * pn : (c + 1) * pn] ** 2)[:, None]
        ).astype(BF16)

        in_maps.append(
            dict(
                xpw=xpw,
                xself=xself,
                idx=idx_w,
                dst=dst_w,
                diso=dis_own_w,
                w1=w1c,
                w2=w2c,
                b1=b1r,
                b2=b2r,
                iota=iota,
                ident=ident,
            )
        )

    meta = dict(seg_len=seg_len, off=off, nidx=nidx, co=co, tot=tot,
                tot_io=tot_io, supers=supers, ni_u=ni_u)
    return in_maps, meta


def build_program_v2(cfg, meta):
    import concourse.bass as bass
    import concourse.mybir as mybir
    import concourse.tile as tile
    import bass_rust as _br
    from contextlib import ExitStack

    f32 = mybir.dt.float32
    bf16 = mybir.dt.bfloat16
    i16 = mybir.dt.int16
    Alu = mybir.AluOpType
    Act = mybir.ActivationFunctionType

    n, pn, tpc, ppad, nb = cfg["N"], cfg["PN"], cfg["TPC"], cfg["PPAD"], cfg["NB"]
    in_c, hid, out_c = cfg["IN"], cfg["HID"], cfg["OUT"]
    seg_len, off = meta["seg_len"], meta["off"]
    nidx, co, tot, supers = meta["nidx"], meta["co"], meta["tot"], meta["supers"]
    tot_io, ni_u = meta["tot_io"], meta["ni_u"]

    import concourse.bacc as bacc

    nq = int(os.environ.get("GCN_NQ", "4"))
    nc = bacc.Bacc(None, num_devices=NCORES, num_swdge_queues=nq)

    apg = bool(os.environ.get("GCN_APG"))
    bs1 = cfg["BS1"]
    if apg:
        xpw_d = nc.dram_tensor("xpw", [nb, 128, bs1, 2], bf16, kind="ExternalInput")
    else:
        xpw_d = nc.dram_tensor(
            "xpw", [nb, 128, tpc, in_c], bf16, kind="ExternalInput"
        )
    xself_d = nc.dram_tensor("xself", [ppad, in_c], bf16, kind="ExternalInput")
    idx_d = nc.dram_tensor("idx", [128, tot_io // 16], i16, kind="ExternalInput")
    dst_d = nc.dram_tensor("dst", [128, tot // 128], f32, kind="ExternalInput")
    diso_d = nc.dram_tensor("diso", [128, tpc], f32, kind="ExternalInput")
    w1_d = nc.dram_tensor("w1", [2, 128, hid], bf16, kind="ExternalInput")
    w2_d = nc.dram_tensor("w2", [2, 128, out_c], bf16, kind="ExternalInput")
    b1_d = nc.dram_tensor("b1", [128, hid], f32, kind="ExternalInput")
    b2_d = nc.dram_tensor("b2", [128, out_c], f32, kind="ExternalInput")
    iota_d = nc.dram_tensor("iota", [128, 128], bf16, kind="ExternalInput")
    ident_d = nc.dram_tensor("ident", [128, 128], bf16, kind="ExternalInput")
    out_d = nc.dram_tensor("out", [ppad, out_c], f32, kind="ExternalOutput")

    tab_w = nc.dram_tensor("tabw", [NCORES * 128, tpc, out_c], bf16,
                           addr_space="Shared")

    chunk_tiles = cfg["CHUNK_TILES"]
    chunks = [
        list(range(i, min(i + chunk_tiles, tpc))) for i in range(0, tpc, chunk_tiles)
    ]

    nreg_cache = {}
    qc = [0]
    stages = os.environ.get("GCN_STAGES", "all")
    skip_mm = os.environ.get("GCN_SKIP_MM")

    with tile.TileContext(nc) as tc, ExitStack() as ctx:
        def nreg(v):
            if v not in nreg_cache:
                nreg_cache[v] = nc.gpsimd.to_reg(v)
            return nreg_cache[v]

        cpool = ctx.enter_context(tc.tile_pool(name="const", bufs=1))
        iota_t = cpool.tile([128, 128], bf16)
        nc.sync.dma_start(iota_t[:], iota_d[:])
        ident_t = cpool.tile([128, 128], bf16)
        nc.sync.dma_start(ident_t[:], ident_d[:])
        w1_t = cpool.tile([128, 2, hid], bf16)
        w2_t = cpool.tile([128, 2, out_c], bf16)
        for k in range(2):
            nc.sync.dma_start(w1_t[:, k, :], w1_d[k])
            nc.sync.dma_start(w2_t[:, k, :], w2_d[k])
        b1_t = cpool.tile([128, hid], f32)
        nc.sync.dma_start(b1_t[:], b1_d[:])
        b2_t = cpool.tile([128, out_c], f32)
        nc.sync.dma_start(b2_t[:], b2_d[:])
        diso_t = cpool.tile([128, tpc], f32)
        nc.sync.dma_start(diso_t[:], diso_d[:])

        dram = ctx.enter_context(tc.tile_pool(name="dram", bufs=1, space="DRAM"))
        mx32_w = dram.tile([128, tpc, hid], f32)
        agg2_w = dram.tile([128, tpc, out_c], f32)
        mx_dm = dram.tile([ppad, hid], bf16)
        o1s_dm = dram.tile([ppad, hid], bf16)
        agin_w = dram.tile([128, tpc, out_c], bf16)

        max_scols = max(
            int(nidx[b][si]) // 128
            for b in range(nb)
            for si in range(len(supers))
        )

        def agg_v2(table_of_b, feat, planes, acc_w, table_dep=None, mode="dma"):
            gathers = []
            acc_prev = {}
            acc_last = {}
            with (
                tc.tile_pool(name="tbl", bufs=2) as tpl,
                tc.tile_pool(name="idxp", bufs=3) as ipool,
                tc.tile_pool(name="msgs", bufs=cfg["MSG_BUFS"]) as mpool,
                tc.tile_pool(name="mse", bufs=4) as mepool,
                tc.tile_pool(name="dstp", bufs=3) as dpool,
                tc.tile_pool(name="oneh", bufs=8) as spool,
                tc.tile_pool(name="stg", bufs=3) as stpool,
                tc.tile_pool(name="psagg", bufs=3, space="PSUM") as pa,
                tc.tile_pool(name="pstr", bufs=4, space="PSUM") as pt,
            ):
                for b in range(nb):
                    if mode == "apg1":
                        tb = tpl.tile([128, bs1, 2], bf16, tag="tbl")
                    elif mode == "apg2":
                        tb = tpl.tile([128, tpc * 128, 1], f32, tag="tbl")
                    else:
                        tb = tpl.tile([128, tpc, feat], bf16, tag="tbl")
                    ld = nc.sync.dma_start(tb[:], table_of_b(b))
                    if table_dep is not None:
                        _br.add_dep_helper(ld.ins, table_dep.ins, sync=True,
                                           reason="tab after AG")
                    for si, s in enumerate(supers):
                        c0 = int(co[b][si])
                        scols = int(nidx[b][si]) // 128
                        if scols == 0:
                            continue
                        it = ipool.tile([128, ni_u // 16], i16, tag="idx")
                        nc.sync.dma_start(
                            it[:], idx_d[:, c0 // 16 : (c0 + ni_u) // 16]
                        )
                        if mode == "apg1":
                            mt = mpool.tile([128, ni_u, 2], bf16, tag="msgs")
                        elif mode == "apg2":
                            mt = mpool.tile([128, ni_u, 1], f32, tag="msgs")
                        else:
                            mt = mpool.tile([128, planes, ni_u], bf16, tag="msgs")
                        if os.environ.get("GCN_SKIP_GATHER"):
                            continue
                        if mode == "apg1":
                            g = nc.gpsimd.ap_gather(
                                mt[:], tb[:], it[:], 128, bs1, 2, ni_u
                            )
                        elif mode == "apg2":
                            g = nc.gpsimd.ap_gather(
                                mt[:], tb[:], it[:], 128, tpc * 128, 1, ni_u
                            )
                        else:
                            g = nc.gpsimd.dma_gather(
                                mt[:],
                                tb[:],
                                it[:],
                                ni_u,
                                nreg(ni_u),
                                feat,
                                transpose=True,
                                sbuf_tokens_per_rank=128,
                                sbuf_free_dim_per_rank=feat * 2,
                                queue_num=qc[0] % nq,
                            )
                            qc[0] += 1
                        gathers.append(g)
                        dt_ = dpool.tile([128, max_scols], f32, tag="dst")
                        nc.sync.dma_start(
                            dt_[:, :scols],
                            dst_d[:, c0 // 128 : c0 // 128 + scols],
                        )
                        st = stpool.tile([128, cfg["TSUP"], feat], f32, tag="stg")
                        for ti, t in enumerate(s):
                            nbt = int(seg_len[t][b]) // 128
                            if nbt == 0 or skip_mm:
                                nc.vector.memset(st[:, ti, :], 0.0)
                                continue
                            ps = pa.tile([128, feat], f32, tag="psagg")
                            for j in range(nbt):
                                mcol = (int(off[t][b]) - c0) // 128 + j
                                ms = mepool.tile([128, feat], bf16, tag="mse")
                                for pl in range(planes):
                                    if mode == "apg1":
                                        src = mt[:, mcol * 128 : (mcol + 1) * 128, pl]
                                    elif mode == "apg2":
                                        f32src = mt[:, mcol * 128 : (mcol + 1) * 128, 0]
                                        bsrc = mepool.tile(
                                            [128, 128], bf16, tag="msecv"
                                        )
                                        nc.vector.tensor_copy(bsrc[:], f32src)
                                        src = bsrc[:]
                                    else:
                                        src = mt[:, pl, mcol * 128 : (mcol + 1) * 128]
                                    ptile = pt.tile([128, 128], bf16, tag="pstr")
                                    nc.tensor.transpose(
                                        ptile[:], src, ident_t[:]
                                    )
                                    nc.scalar.activation(
                                        ms[:, pl * 128 : (pl + 1) * 128],
                                        ptile[:],
                                        Act.Copy,
                                    )
                                S = spool.tile([128, 128], bf16, tag="oneh")
                                nc.vector.tensor_scalar(
                                    S[:], iota_t[:], dt_[:, mcol : mcol + 1],
                                    None, Alu.is_equal,
                                )
                                nc.tensor.matmul(
                                    ps[:], S[:], ms[:],
                                    start=(j == 0), stop=(j == nbt - 1),
                                )
                            nc.vector.tensor_copy(st[:, ti, :], ps[:])
                        if os.environ.get("GCN_SKIP_ACC"):
                            continue
                        t0 = s[0]
                        acc = nc.gpsimd.dma_start(
                            acc_w[:, t0 : t0 + len(s), :],
                            st[:, : len(s), :],
                            accum_op=(Alu.bypass if b == 0 else Alu.add),
                        )
                        if si in acc_prev:
                            _br.add_dep_helper(acc.ins, acc_prev[si].ins,
                                               sync=True, reason="acc order")
                        acc_prev[si] = acc
                        for t in s:
                            acc_last[t] = acc
            return gathers, acc_last

        # ---------------- layer-1 aggregation
        _, acc1_last = agg_v2(lambda b: xpw_d[b], in_c, 2, mx32_w,
                              mode="apg1" if apg else "dma")

        # ---------------- convert: mx = diso * mx32 + xself   (bf16)
        if stages != "l1":
            with (
                tc.tile_pool(name="cva", bufs=3) as cvp,
                tc.tile_pool(name="cvx", bufs=3) as xsp,
                tc.tile_pool(name="cvo", bufs=3) as mxp,
            ):
                for t in range(tpc):
                    a32 = cvp.tile([128, hid], f32, tag="cva")
                    rd = nc.sync.dma_start(a32[:], mx32_w[:, t, :])
                    _br.add_dep_helper(rd.ins, acc1_last[t].ins, sync=True,
                                       reason="mx32 done")
                    xst = xsp.tile([128, in_c], bf16, tag="cvx")
                    nc.sync.dma_start(xst[:], xself_d[t * 128 : (t + 1) * 128, :])
                    mxt = mxp.tile([128, hid], bf16, tag="cvo")
                    nc.vector.scalar_tensor_tensor(
                        mxt[:], a32[:], diso_t[:, t : t + 1], xst[:],
                        Alu.mult, Alu.add,
                    )
                    nc.sync.dma_start(mx_dm[t * 128 : (t + 1) * 128, :], mxt[:])

        # ---------------- dense W1 + relu + dis scale
        if stages not in ("l1", "l1c"):
            with (
                tc.tile_pool(name="mxT", bufs=4) as tp,
                tc.tile_pool(name="tadd", bufs=3) as tap,
                tc.tile_pool(name="o1", bufs=3) as o1p,
                tc.tile_pool(name="psB", bufs=2, space="PSUM") as pb,
            ):
                for ch in chunks:
                    r0, rc = ch[0] * 128, len(ch) * 128
                    mxT = []
                    for k in range(2):
                        mt_ = tp.tile([128, chunk_tiles * 128], bf16, tag="mxT")
                        nc.sync.dma_start_transpose(
                            mt_[:, :rc], mx_dm[r0 : r0 + rc, k * 128 : (k + 1) * 128]
                        )
                        mxT.append(mt_)
                    for ri, t in enumerate(ch):
                        ps = pb.tile([128, hid], f32, tag="psB")
                        for k in range(2):
                            nc.tensor.matmul(
                                ps[:],
                                mxT[k][:, ri * 128 : (ri + 1) * 128],
                                w1_t[:, k, :],
                                start=(k == 0),
                                stop=(k == 1),
                            )
                        tt = tap.tile([128, hid], f32, tag="tadd")
                        nc.vector.tensor_add(tt[:], ps[:], b1_t[:])
                        o1 = o1p.tile([128, hid], bf16, tag="o1")
                        nc.scalar.activation(
                            o1[:], tt[:], Act.Relu, scale=diso_t[:, t : t + 1]
                        )
                        nc.sync.dma_start(o1s_dm[t * 128 : (t + 1) * 128, :], o1[:])

        # ---------------- dense W2 -> wrapped AllGather input
        if stages not in ("l1", "l1c", "l1b"):
            with (
                tc.tile_pool(name="o1T", bufs=4) as tp2,
                tc.tile_pool(name="h2", bufs=3) as h2p,
                tc.tile_pool(name="psC", bufs=2, space="PSUM") as pc,
            ):
                for ch in chunks:
                    r0, rc = ch[0] * 128, len(ch) * 128
                    o1T = []
                    for k in range(2):
                        ot = tp2.tile([128, chunk_tiles * 128], bf16, tag="o1T")
                        nc.sync.dma_start_transpose(
                            ot[:, :rc], o1s_dm[r0 : r0 + rc, k * 128 : (k + 1) * 128]
                        )
                        o1T.append(ot)
                    for ri, t in enumerate(ch):
                        ps = pc.tile([128, out_c], f32, tag="psC")
                        for k in range(2):
                            nc.tensor.matmul(
                                ps[:],
                                o1T[k][:, ri * 128 : (ri + 1) * 128],
                                w2_t[:, k, :],
                                start=(k == 0),
                                stop=(k == 1),
                            )
                        h2 = h2p.tile([128, out_c], bf16, tag="h2")
                        nc.vector.tensor_copy(h2[:], ps[:])
                        nc.sync.dma_start(agin_w[:, t, :], h2[:])

        # ---------------- AllGather (wrapped tables)
        if stages not in ("l1", "l1c", "l1b", "l1bc"):
            cc = nc.gpsimd.collective_compute(
                "AllGather",
                mybir.AluOpType.bypass,
                replica_groups=[list(range(NCORES))],
                ins=[agin_w[:, :, :].opt()],
                outs=[tab_w.ap().opt()],
            )

        # ---------------- layer-2 aggregation + epilogue
        if stages == "all":
            _, acc2_last = agg_v2(
                lambda b: tab_w[b * 128 : (b + 1) * 128, :, :],
                out_c, 1, agg2_w, table_dep=cc,
            )

            with (
                tc.tile_pool(name="ea", bufs=3) as eap,
                tc.tile_pool(name="eh", bufs=3) as ehp,
                tc.tile_pool(name="ew", bufs=3) as ewp,
                tc.tile_pool(name="eo", bufs=3) as eop,
            ):
                for t in range(tpc):
                    a2 = eap.tile([128, out_c], f32, tag="ea")
                    rd = nc.sync.dma_start(a2[:], agg2_w[:, t, :])
                    _br.add_dep_helper(rd.ins, acc2_last[t].ins, sync=True,
                                       reason="agg2 done")
                    h2o = ehp.tile([128, out_c], bf16, tag="eh")
                    nc.sync.dma_start(h2o[:], agin_w[:, t, :])
                    wt = ewp.tile([128, out_c], f32, tag="ew")
                    nc.vector.scalar_tensor_tensor(
                        wt[:], h2o[:], diso_t[:, t : t + 1], b2_t[:],
                        Alu.mult, Alu.add,
                    )
                    o2 = eop.tile([128, out_c], f32, tag="eo")
                    nc.vector.scalar_tensor_tensor(
                        o2[:], a2[:], diso_t[:, t : t + 1], wt[:],
                        Alu.mult, Alu.add,
                    )
                    nc.sync.dma_start(out_d[t * 128 : (t + 1) * 128, :], o2[:])

        if stages != "all":
            nc.sync.dma_start(out_d[0:128, :], b2_t[:])

    nc.compile()
    return nc


# ---------------------------------------------------------------- entry point
def kernel(x, edge_index, W1, b1, W2, b2):
    from concourse.bass_utils import run_bass_kernel_spmd

    if os.environ.get("GCN_V2"):
        cfg = make_cfg(N, IN_C, HID, OUT_C, msg_bufs=4, nb=8)
        in_maps, meta = prep_inputs_v2(cfg, x, edge_index, W1, b1, W2, b2)
        nc = build_program_v2(cfg, meta)
    else:
        os.environ.setdefault("GCN_NQ", "4")
        cfg = CFG
        in_maps, meta = prep_inputs(cfg, x, edge_index, W1, b1, W2, b2)
        nc = build_program(cfg, meta)
    res = run_bass_kernel_spmd(nc, in_maps, core_ids=list(range(NCORES)))
    outs = [r["out"][: cfg["PN"]] for r in res.results]
    return np.concatenate(outs, axis=0).astype(np.float32)

